# revision 1
# baseline (speedup 1.0000x reference)
# Trainium2 Bass kernel for nn_DiversityLoss (segment_reduce).
#
# reference:
#   sums   = segment_sum(embeddings, labels, C)        # [C, D]
#   counts = segment_sum(ones, labels, C)              # [C]
#   return -mean(var(sums / counts, axis=0, ddof=1))
#
# Strategy (data-parallel across 8 NeuronCores):
#   - Shard N=1M rows into 8 shards of 125k rows.
#   - On each core, compute the per-class partial sums of its shard with a
#     one-hot matmul on the Tensor engine:
#        for each 128-row tile t (977 tiles/core):
#           onehot[p, c] = (label[row p of t] == c)      (DVE is_equal vs iota)
#           psum[D, C]  += emb_tile[K=128rows, M=128D]^T @ onehot[K=128, N=C]
#     accumulated in PSUM (fp32) across all tiles, then flushed to DRAM.
#   - Host: sum the 8 partial [D, C] outputs, counts via bincount on the
#     labels (0.2% of input bytes), then means/variance in float64.
#
# Measured steady state (NTFF profile): 423 ns/tile with zero pipeline
# stalls, which is the PE floor — the moving one-hot streams 2x500
# columns/tile at 1 column/cycle @ 2.4 GHz (the 1000-wide encoding is
# rank-bound for exact per-class sums); the DVE one-hot build (~478 ns,
# overlapped to 423) paces evenly with it. HW exec time ~433 us/core on a
# cool chip (the device throttles ~20% chip-wide under co-tenant load and
# recovers after a few minutes idle).
#
# Layout prep on host (pure layout/dtype glue, no reduction math):
#   - embeddings cast fp32->fp16 and laid out [p, t, d]-contiguous per core so
#     each SBUF partition's DMA stream is fully contiguous.
#   - labels as fp32 in [p, t] layout (tensor_scalar scalars must be fp32);
#     pad rows use label -1, which never matches the iota, and emb 0.

import numpy as np

N = 1_000_000
D = 128
C = 1000
CORES = 8
NSH = N // CORES  # 125_000 rows per core
TILES = 977  # 977 * 128 = 125_056 padded rows per core
G = 49  # row-tiles per DMA chunk
CPAD_HALF = 500
CPAD = 1000  # exact class count; pad labels are -1 (never match)

# test.py can flip this before calling kernel() to capture a profile; the
# BassKernelResults of the last run is stored in LAST_RESULT either way.
TRACE = False
TRACE_KWARGS = {}
LAST_RESULT = None

_cached_nc = None


def _build_module():
    import concourse.mybir as mybir
    import concourse.tile as tile
    from concourse import bacc

    f16 = mybir.dt.float16
    f32 = mybir.dt.float32

    nc = bacc.Bacc(
        "TRN2",
        target_bir_lowering=False,
        debug=False,
        enable_asserts=False,
        num_devices=CORES,
    )
    emb_d = nc.dram_tensor("emb", [128, TILES * D], f16, kind="ExternalInput")
    lab_d = nc.dram_tensor("lab", [128, TILES], f32, kind="ExternalInput")
    out_d = nc.dram_tensor("out", [128, CPAD], f32, kind="ExternalOutput")

    with tile.TileContext(nc) as tc:
        with (
            tc.tile_pool(name="consts", bufs=1) as consts,
            tc.tile_pool(name="ebuf", bufs=6) as ebuf,
            tc.tile_pool(name="obuf", bufs=12) as obuf,
            tc.tile_pool(name="psum", bufs=1, space="PSUM") as psum,
            tc.tile_pool(name="outb", bufs=1) as outb,
        ):
            lab_t = consts.tile([128, TILES], f32)
            iota_t = consts.tile([128, CPAD], mybir.dt.int16)

            # iota generated on the otherwise-idle GpSimd engine: ready before
            # the DMA pipe spins up, so tile 0 is gated only by the tiny
            # first-labels DMA.
            nc.gpsimd.iota(iota_t[:], [[1, CPAD]], channel_multiplier=0)
            nc.sync.dma_start(out=lab_t[:, 0:32], in_=lab_d[:, 0:32])

            # Two PSUM banks accumulate [D=128, C=1000] fp32 across all tiles.
            psA = psum.tile([128, CPAD_HALF], f32)
            psB = psum.tile([128, CPAD_HALF], f32)

            # First chunks are small so compute starts as soon as possible.
            splits = [0, 8, 32]
            while splits[-1] < TILES:
                splits.append(min(splits[-1] + G, TILES))
            for ch in range(len(splits) - 1):
                t0, t1 = splits[ch], splits[ch + 1]
                et = ebuf.tile([128, G * D], f16, tag="et")
                nc.sync.dma_start(
                    out=et[:, 0 : (t1 - t0) * D],
                    in_=emb_d[:, t0 * D : t1 * D],
                )
                if ch == 1:
                    # Bulk of the labels, behind the first two chunks in the
                    # DMA queue (not needed until tile 32).
                    nc.sync.dma_start(
                        out=lab_t[:, 32:TILES], in_=lab_d[:, 32:TILES]
                    )
                for i in range(t1 - t0):
                    t = t0 + i
                    oh = obuf.tile([128, CPAD], f16)
                    nc.vector.tensor_scalar(
                        out=oh[:],
                        in0=iota_t[:],
                        scalar1=lab_t[:, t : t + 1],
                        scalar2=None,
                        op0=mybir.AluOpType.is_equal,
                    )
                    nc.tensor.matmul(
                        psA[:],
                        lhsT=et[:, i * D : (i + 1) * D],
                        rhs=oh[:, 0:CPAD_HALF],
                        start=(t == 0),
                        stop=(t == TILES - 1),
                    )
                    nc.tensor.matmul(
                        psB[:],
                        lhsT=et[:, i * D : (i + 1) * D],
                        rhs=oh[:, CPAD_HALF:CPAD],
                        start=(t == 0),
                        stop=(t == TILES - 1),
                    )

            out_t = outb.tile([128, CPAD], f32)
            nc.scalar.copy(out=out_t[:, 0:CPAD_HALF], in_=psA[:])
            nc.vector.tensor_copy(out=out_t[:, CPAD_HALF:CPAD], in_=psB[:])
            nc.sync.dma_start(
                out=out_d[:, 0:CPAD_HALF], in_=out_t[:, 0:CPAD_HALF]
            )
            nc.sync.dma_start(
                out=out_d[:, CPAD_HALF:CPAD], in_=out_t[:, CPAD_HALF:CPAD]
            )

    nc.compile()
    return nc


def _prep_inputs(embeddings, labels):
    embeddings = np.asarray(embeddings)
    labels = np.asarray(labels).astype(np.int64)

    in_maps = []
    for s in range(CORES):
        e = embeddings[s * NSH : (s + 1) * NSH]
        l = labels[s * NSH : (s + 1) * NSH]

        ep = np.zeros((TILES * 128, D), dtype=np.float16)
        ep[:NSH] = e.astype(np.float16)
        lp = np.full((TILES * 128,), -1.0, dtype=np.float32)
        lp[:NSH] = l.astype(np.float32)

        emb_t = np.ascontiguousarray(
            ep.reshape(TILES, 128, D).transpose(1, 0, 2)
        ).reshape(128, TILES * D)
        lab_t = np.ascontiguousarray(lp.reshape(TILES, 128).T)
        in_maps.append({"emb": emb_t, "lab": lab_t})
    return in_maps


def kernel(embeddings, labels):
    global _cached_nc, LAST_RESULT
    from concourse.bass_utils import run_bass_kernel_spmd

    if _cached_nc is None:
        _cached_nc = _build_module()
    nc = _cached_nc

    in_maps = _prep_inputs(embeddings, labels)
    res = run_bass_kernel_spmd(
        nc,
        in_maps,
        core_ids=list(range(CORES)),
        trace=TRACE,
        **TRACE_KWARGS,
    )
    LAST_RESULT = res

    acc = np.zeros((128, CPAD), dtype=np.float64)
    for r in res.results:
        acc += r["out"].astype(np.float64)
    sums = acc.T[:C]  # [C, D]

    labels64 = np.asarray(labels).astype(np.int64)
    counts = np.bincount(labels64, minlength=C).astype(np.float64)

    means = sums / counts[:, None]
    mu = means.mean(axis=0)
    var = ((means - mu) ** 2).sum(axis=0) / (C - 1)
    return np.float32(-var.mean())



# revision 3
# speedup vs baseline: 6.2587x; 6.2587x over previous
# Trainium2 Bass kernel for nn_DiversityLoss (segment_reduce).
#
# reference:
#   sums   = segment_sum(embeddings, labels, C)        # [C, D]
#   counts = segment_sum(ones, labels, C)              # [C]
#   return -mean(var(sums / counts, axis=0, ddof=1))
#
# Strategy (v2, sorted class-pure tiles; baseline one-hot matmul was PE-bound
# at ~1000 one-hot columns per 128-row tile = 433 us):
#   - Host sorts rows by label, pads each class to a multiple of 128 rows
#     (~6.5% padding), quantizes embeddings to fp8e4 (measured 9.7e-4 final
#     rel err vs the 2e-2 gate), and deals classes 125-per-core so that every
#     core has an identical per-position tile-count sequence (SPMD shares one
#     program across the 8 cores).
#   - Each 128-row tile is class-pure, so its segment-sum is a plain matmul
#     with a tiny [K=128, M=32] one-hot stationary (LDW ~27 ns, MM N=128
#     ~53 ns) that routes the tile's row-sum into the class's PSUM partition.
#     tile_position rotates over the 4 PE column groups so consecutive tiles
#     overlap inside the array; each (colgroup, class-layer) pair is its own
#     PSUM accumulation region in a single [128, 512] fp32 bank.
#   - Host sums the 4 column-group replicas per class, divides by exact
#     bincount counts, and does the variance in float64.
#   - Device reads 16 KB per tile (fp8): ~17 MB/core -> memory-bound at
#     ~358 GB/s HBM per core.

import numpy as np
import ml_dtypes

D = 128
C = 1000
CORES = 8
CPC = C // CORES  # 125 classes per core
G = 128  # tiles per DMA chunk

TRACE = False
TRACE_KWARGS = {}
LAST_RESULT = None

_cache = {}


def _build_module(T_pos):
    import concourse.mybir as mybir
    import concourse.tile as tile
    from concourse import bacc

    f8 = mybir.dt.float8e4
    f32 = mybir.dt.float32

    NT = int(sum(T_pos))

    nc = bacc.Bacc(
        "TRN2",
        target_bir_lowering=False,
        debug=False,
        enable_asserts=False,
        num_devices=CORES,
    )
    emb_d = nc.dram_tensor("emb", [128, NT * D], f8, kind="ExternalInput")
    w_d = nc.dram_tensor("w", [128, 32 * 32], f8, kind="ExternalInput")
    out_d = nc.dram_tensor("out", [128, 512], f32, kind="ExternalOutput")

    # tile t -> class position p
    tiles = [p for p in range(CPC) for _ in range(T_pos[p])]

    with tile.TileContext(nc) as tc:
        with (
            tc.tile_pool(name="consts", bufs=1) as consts,
            tc.tile_pool(name="ebuf", bufs=5) as ebuf,
            tc.tile_pool(name="psum", bufs=1, space="PSUM") as psum,
            tc.tile_pool(name="outb", bufs=1) as outb,
        ):
            w_t = consts.tile([128, 32 * 32], f8)
            nc.sync.dma_start(out=w_t[:], in_=w_d[:])

            ps = psum.tile([128, 512], f32)

            # First chunks are small so compute starts as soon as possible.
            splits = [0, 8, 40]
            while splits[-1] < NT:
                splits.append(min(splits[-1] + G, NT))
            t = 0
            for ch in range(len(splits) - 1):
                t0, t1 = splits[ch], splits[ch + 1]
                et = ebuf.tile([128, G * D], f8, tag="et")
                nc.sync.dma_start(
                    out=et[:, 0 : (t1 - t0) * D],
                    in_=emb_d[:, t0 * D : t1 * D],
                )
                for i in range(t1 - t0):
                    p = tiles[t]
                    r = t % 4
                    l = p // 32
                    j32 = p % 32
                    key = (r, l)
                    nc.tensor.matmul(
                        ps[32 * r : 32 * r + 32, 128 * l : 128 * (l + 1)],
                        lhsT=w_t[:, 32 * j32 : 32 * j32 + 32],
                        rhs=et[:, i * D : (i + 1) * D],
                        start=(first[key] == t),
                        stop=(last[key] == t),
                        tile_position=(0, 32 * r),
                    )
                    t += 1

            out_t = outb.tile([128, 512], f32)
            nc.scalar.copy(out=out_t[:, 0:256], in_=ps[:, 0:256])
            nc.vector.tensor_copy(out=out_t[:, 256:512], in_=ps[:, 256:512])
            nc.sync.dma_start(out=out_d[:, 0:256], in_=out_t[:, 0:256])
            nc.sync.dma_start(out=out_d[:, 256:512], in_=out_t[:, 256:512])

    nc.compile()
    return nc


def _schedule(counts):
    T_c = -(-counts // 128)  # ceil
    rank = np.argsort(-T_c, kind="stable")  # class ids, tile count descending
    T_pos = T_c[rank[np.arange(CPC) * 8]]  # max of each octet
    return rank, tuple(int(x) for x in T_pos)


def kernel(embeddings, labels):
    global LAST_RESULT
    from concourse.bass_utils import run_bass_kernel_spmd

    embeddings = np.asarray(embeddings)
    labels = np.asarray(labels).astype(np.int64)
    N = labels.shape[0]

    counts = np.bincount(labels, minlength=C)
    rank, T_pos = _schedule(counts)
    NT = int(sum(T_pos))

    key = T_pos
    if key not in _cache:
        _cache[key] = _build_module(list(T_pos))
    nc = _cache[key]

    # ---- host layout: sorted, class-padded, per-core ----
    embq = embeddings.astype(ml_dtypes.float8_e4m3)
    embq_ext = np.zeros((N + 1, D), dtype=ml_dtypes.float8_e4m3)
    embq_ext[:N] = embq
    order = np.argsort(labels, kind="stable")
    cls_start = np.zeros(C + 1, dtype=np.int64)
    np.cumsum(counts, out=cls_start[1:])

    slot_base = np.zeros(CPC + 1, dtype=np.int64)
    np.cumsum(np.asarray(T_pos, dtype=np.int64) * 128, out=slot_base[1:])

    w = np.zeros((128, 32 * 32), dtype=ml_dtypes.float8_e4m3)
    w[:, 33 * np.arange(32)] = 1.0

    in_maps = []
    for k in range(CORES):
        idx = np.full(NT * 128, N, dtype=np.int64)
        for p in range(CPC):
            c = rank[8 * p + k]
            n = counts[c]
            idx[slot_base[p] : slot_base[p] + n] = order[
                cls_start[c] : cls_start[c] + n
            ]
        ec = embq_ext[idx]  # [NT*128, D] fp8
        emb_t = np.ascontiguousarray(
            ec.reshape(NT, 128, D).transpose(1, 0, 2)
        ).reshape(128, NT * D)
        in_maps.append({"emb": emb_t, "w": w})

    res = run_bass_kernel_spmd(
        nc,
        in_maps,
        core_ids=list(range(CORES)),
        trace=TRACE,
        **TRACE_KWARGS,
    )
    LAST_RESULT = res

    # ---- host combine: sum 4 colgroup replicas, then means/variance ----
    sums = np.zeros((C, D), dtype=np.float64)
    for k in range(CORES):
        o = res.results[k]["out"].astype(np.float64)
        # [r=4, j32=32, l=4, d=128] -> sum over r -> [l, j32, d] -> [p, d]
        s_all = o.reshape(4, 32, 4, 128).sum(axis=0).transpose(1, 0, 2)
        s_all = s_all.reshape(CPC + 3, D)[:CPC]
        sums[rank[np.arange(CPC) * 8 + k]] = s_all
    means = sums / counts[:, None]
    mu = means.mean(axis=0)
    var = ((means - mu) ** 2).sum(axis=0) / (C - 1)
    return np.float32(-var.mean())


# revision 6
# speedup vs baseline: 6.2748x; 1.0026x over previous
# Trainium2 Bass kernel for nn_DiversityLoss (segment_reduce).
#
# reference:
#   sums   = segment_sum(embeddings, labels, C)        # [C, D]
#   counts = segment_sum(ones, labels, C)              # [C]
#   return -mean(var(sums / counts, axis=0, ddof=1))
#
# Strategy (v2, sorted class-pure tiles; baseline one-hot matmul was PE-bound
# at ~1000 one-hot columns per 128-row tile = 433 us):
#   - Host sorts rows by label, pads each class to a multiple of 128 rows
#     (~6.5% padding), quantizes embeddings to fp8e4 (measured 9.7e-4 final
#     rel err vs the 2e-2 gate), and deals classes 125-per-core so that every
#     core has an identical per-position tile-count sequence (SPMD shares one
#     program across the 8 cores).
#   - Each 128-row tile is class-pure, so its segment-sum is a plain matmul
#     with a tiny [K=128, M=32] one-hot stationary (LDW ~27 ns, MM N=128
#     ~53 ns) that routes the tile's row-sum into the class's PSUM partition.
#     tile_position rotates over the 4 PE column groups so consecutive tiles
#     overlap inside the array; each (colgroup, class-layer) pair is its own
#     PSUM accumulation region in a single [128, 512] fp32 bank.
#   - Host sums the 4 column-group replicas per class, divides by exact
#     bincount counts, and does the variance in float64.
#   - Device reads 16 KB per tile (fp8): ~17 MB/core -> memory-bound at
#     ~358 GB/s HBM per core.

import numpy as np
import ml_dtypes

D = 128
C = 1000
CORES = 8
CPC = C // CORES  # 125 classes per core
G = 64  # tiles per DMA chunk
BUFS = 10  # chunk buffers in flight

TRACE = False
TRACE_KWARGS = {}
LAST_RESULT = None

_cache = {}


def _build_module(T_pos):
    import concourse.mybir as mybir
    import concourse.tile as tile
    from concourse import bacc

    f8 = mybir.dt.float8e4
    f32 = mybir.dt.float32

    NT = int(sum(T_pos))

    nc = bacc.Bacc(
        "TRN2",
        target_bir_lowering=False,
        debug=False,
        enable_asserts=False,
        num_devices=CORES,
    )
    emb_d = nc.dram_tensor("emb", [128, NT * D], f8, kind="ExternalInput")
    w_d = nc.dram_tensor("w", [128, 32 * 32], f8, kind="ExternalInput")
    out_d = nc.dram_tensor("out", [128, 512], f32, kind="ExternalOutput")

    # tile t -> class position p; region = (colgroup r, class layer l)
    tiles = [p for p in range(CPC) for _ in range(T_pos[p])]
    first = {}
    last = {}
    for t, p in enumerate(tiles):
        key = (t % 4, p // 32)
        first.setdefault(key, t)
        last[key] = t

    # last tile index of each class layer (for early psum flush)
    layer_last = {}
    for t, p in enumerate(tiles):
        layer_last[p // 32] = t

    with tile.TileContext(nc) as tc:
        with (
            tc.tile_pool(name="consts", bufs=1) as consts,
            tc.tile_pool(name="ebuf", bufs=BUFS) as ebuf,
            tc.tile_pool(name="psum", bufs=1, space="PSUM") as psum,
            tc.tile_pool(name="outb", bufs=1) as outb,
        ):
            w_t = consts.tile([128, 32 * 32], f8)
            nc.sync.dma_start(out=w_t[:], in_=w_d[:])

            ps = psum.tile([128, 512], f32)
            out_t = outb.tile([128, 512], f32)

            flush_after = {layer_last[l]: l for l in layer_last}

            # First chunks are small so compute starts as soon as possible.
            splits = [0, 4, 16, 40]
            while splits[-1] < NT:
                splits.append(min(splits[-1] + G, NT))
            t = 0
            for ch in range(len(splits) - 1):
                t0, t1 = splits[ch], splits[ch + 1]
                et = ebuf.tile([128, G * D], f8, tag="et")
                nc.sync.dma_start(
                    out=et[:, 0 : (t1 - t0) * D],
                    in_=emb_d[:, t0 * D : t1 * D],
                )
                for i in range(t1 - t0):
                    p = tiles[t]
                    r = t % 4
                    l = p // 32
                    j32 = p % 32
                    key = (r, l)
                    nc.tensor.matmul(
                        ps[32 * r : 32 * r + 32, 128 * l : 128 * (l + 1)],
                        lhsT=w_t[:, 32 * j32 : 32 * j32 + 32],
                        rhs=et[:, i * D : (i + 1) * D],
                        start=(first[key] == t),
                        stop=(last[key] == t),
                        tile_position=(0, 32 * r),
                    )
                    if t in flush_after:
                        # this class layer is complete: flush its psum
                        # columns while later layers keep accumulating
                        l2 = flush_after[t]
                        nc.vector.tensor_copy(
                            out=out_t[:, 128 * l2 : 128 * (l2 + 1)],
                            in_=ps[:, 128 * l2 : 128 * (l2 + 1)],
                        )
                        nc.sync.dma_start(
                            out=out_d[:, 128 * l2 : 128 * (l2 + 1)],
                            in_=out_t[:, 128 * l2 : 128 * (l2 + 1)],
                        )
                    t += 1

    nc.compile()
    return nc


def _schedule(counts):
    T_c = -(-counts // 128)  # ceil
    rank = np.argsort(-T_c, kind="stable")  # class ids, tile count descending
    T_pos = T_c[rank[np.arange(CPC) * 8]]  # max of each octet
    return rank, tuple(int(x) for x in T_pos)


def kernel(embeddings, labels):
    global LAST_RESULT
    from concourse.bass_utils import run_bass_kernel_spmd

    embeddings = np.asarray(embeddings)
    labels = np.asarray(labels).astype(np.int64)
    N = labels.shape[0]

    counts = np.bincount(labels, minlength=C)
    rank, T_pos = _schedule(counts)
    NT = int(sum(T_pos))

    key = T_pos
    if key not in _cache:
        _cache[key] = _build_module(list(T_pos))
    nc = _cache[key]

    # ---- host layout: sorted, class-padded, per-core ----
    embq = embeddings.astype(ml_dtypes.float8_e4m3)
    embq_ext = np.zeros((N + 1, D), dtype=ml_dtypes.float8_e4m3)
    embq_ext[:N] = embq
    order = np.argsort(labels, kind="stable")
    cls_start = np.zeros(C + 1, dtype=np.int64)
    np.cumsum(counts, out=cls_start[1:])

    slot_base = np.zeros(CPC + 1, dtype=np.int64)
    np.cumsum(np.asarray(T_pos, dtype=np.int64) * 128, out=slot_base[1:])

    w = np.zeros((128, 32 * 32), dtype=ml_dtypes.float8_e4m3)
    w[:, 33 * np.arange(32)] = 1.0

    in_maps = []
    for k in range(CORES):
        idx = np.full(NT * 128, N, dtype=np.int64)
        for p in range(CPC):
            c = rank[8 * p + k]
            n = counts[c]
            idx[slot_base[p] : slot_base[p] + n] = order[
                cls_start[c] : cls_start[c] + n
            ]
        ec = embq_ext[idx]  # [NT*128, D] fp8
        emb_t = np.ascontiguousarray(
            ec.reshape(NT, 128, D).transpose(1, 0, 2)
        ).reshape(128, NT * D)
        in_maps.append({"emb": emb_t, "w": w})

    res = run_bass_kernel_spmd(
        nc,
        in_maps,
        core_ids=list(range(CORES)),
        trace=TRACE,
        **TRACE_KWARGS,
    )
    LAST_RESULT = res

    # ---- host combine: sum 4 colgroup replicas, then means/variance ----
    sums = np.zeros((C, D), dtype=np.float64)
    for k in range(CORES):
        o = res.results[k]["out"].astype(np.float64)
        # [r=4, j32=32, l=4, d=128] -> sum over r -> [l, j32, d] -> [p, d]
        s_all = o.reshape(4, 32, 4, 128).sum(axis=0).transpose(1, 0, 2)
        s_all = s_all.reshape(CPC + 3, D)[:CPC]
        sums[rank[np.arange(CPC) * 8 + k]] = s_all
    means = sums / counts[:, None]
    mu = means.mean(axis=0)
    var = ((means - mu) ** 2).sum(axis=0) / (C - 1)
    return np.float32(-var.mean())
